# revision 33
# baseline (speedup 1.0000x reference)
"""Bass/Trainium2 kernel for nn_BaselineLSTM (B=2048, T=512, H=128, twin=256).

Strategy (v2):
  - Data-parallel: batch 2048 -> 8 cores x 256; each core runs 2 interleaved
    chunks of 128 batch (pipelining hides per-step cross-engine latency).
  - State kept transposed: h/c = [H=128 partitions, batch free]; state
    variables are scaled: hT = h/2, cT = 2c, so that every tanh can be
    computed as a sigmoid and all fix-up constants fold into weights:
      tanh(x) = 2*sigmoid(2x) - 1.
  - ONE sigmoid ACT per chunk-step covers all four gates [i|f|o|g]: the
    g-block rows of the stationary weights are pre-scaled so the matmul
    emits 2*pre_g there; a second small sigmoid covers sigma(cT)=sigma(2c).
  - Input + bias enter via ONE K=8 (phase P) / K=4 (phase H) matmul with a
    block-diagonal rhs (phase P rhs streamed from DRAM, phase H rhs static),
    accumulated into the gates PSUM bank before the 4 recurrent matmuls.
  - Cell update on DVE only (gpsimd is pathologically slow for elementwise):
      t2 = sf*cT;  u = (s2g-0.5)*si;  cT = 4u + t2       (scalar_tensor_tensor)
      hT = (sigma(cT)-0.5)*so                             (scalar_tensor_tensor)
  - fp16 everywhere on-chip (not bf16): the 2*sigmoid(2x)-1 rewrite loses
    absolute precision near 0.5 in bf16; fp16's 10 mantissa bits restore it,
    and fp16 keeps the DVE 2x/4x packed modes.
  - Predictions p_t = (2*W_out) hT_t (+ b_out on host): hT kept in a 4-slot
    ring; one shifted-stationary matmul per 4 steps accumulates 128 steps
    into one PSUM bank, flushed to DRAM per 128-step epoch.
"""

import functools

import numpy as np

import concourse.bacc as bacc
import concourse.tile as tile
from concourse import mybir
from concourse.bass_utils import run_bass_kernel_spmd

F32 = mybir.dt.float32
FP16 = mybir.dt.float16
AF = mybir.ActivationFunctionType
OP = mybir.AluOpType

H = 128          # hidden
NCORES = 8
BS = 256         # batch per core
BC = 128         # batch per chunk
NCHUNK = 2
BLK = 32         # xq steps per DMA block

# kernel gate order == pytorch order (i, f, g, o): sigma(i,f,g) is one
# contiguous on-chain activation; sigma(o) is separate and off-chain (o is
# first needed only after tanh(c)).
_PERM = np.arange(512)
# g-gate rows doubled so the matmul emits 2*pre_g for the
# tanh(x) = 2*sigmoid(2x)-1 rewrite; cT state = 2c.
_SCALE = np.repeat([1.0, 1.0, 2.0, 1.0], 128)
_SCALE_B = _SCALE


def _build_body(tc, d, NP, NH):
    nc = tc.nc
    NT = NP + NH
    NBLK = (NP + BLK - 1) // BLK

    import contextlib
    with contextlib.ExitStack() as ctx:
        consts = ctx.enter_context(tc.tile_pool(name="consts", bufs=1))
        state = ctx.enter_context(tc.tile_pool(name="state", bufs=1))
        spool = ctx.enter_context(tc.tile_pool(name="sig", bufs=3))
        wpool = ctx.enter_context(tc.tile_pool(name="work", bufs=3))
        xpool = ctx.enter_context(tc.tile_pool(name="xq", bufs=2))
        gpool = ctx.enter_context(tc.tile_pool(name="gates", bufs=2, space="PSUM"))
        ppool = ctx.enter_context(tc.tile_pool(name="ppsum", bufs=1, space="PSUM"))

        # ---- constants to SBUF
        whhT_p = consts.tile([H, 4 * H], FP16, tag="whhT_p")
        whhT_h = consts.tile([H, 4 * H], FP16, tag="whhT_h")
        whhT_hn = consts.tile([H, 4 * H], FP16, tag="whhT_hn")
        bp8 = consts.tile([8, H], FP16, tag="bp8")
        bh4 = consts.tile([4, H], FP16, tag="bh4")
        ones4 = consts.tile([4, 4 * BC], FP16, tag="ones4")
        woutZ = consts.tile([H, 2 * H], FP16, tag="woutZ")
        nc.sync.dma_start(out=whhT_p, in_=d["whhT_p"])
        nc.sync.dma_start(out=whhT_h, in_=d["whhT_h"])
        nc.sync.dma_start(out=whhT_hn, in_=d["whhT_hn"])
        nc.sync.dma_start(out=bp8, in_=d["bp8"])
        nc.sync.dma_start(out=bh4, in_=d["bh4"])
        nc.sync.dma_start(out=ones4, in_=d["ones4"])
        nc.sync.dma_start(out=woutZ, in_=d["woutZ"])

        # ---- state
        hist = []
        cT = []
        for ch in range(NCHUNK):
            hh = state.tile([H, 8 * BC], FP16, tag=f"hist{ch}")
            c = state.tile([H, BC], FP16, tag=f"cT{ch}")
            nc.vector.memset(hh, 0.0)
            nc.vector.memset(c, 0.0)
            hist.append(hh)
            cT.append(c)

        # ---- xq stream (phase P block-diag rhs), double buffered
        xtiles = [[None] * NBLK for _ in range(NCHUNK)]

        def fetch(blk):
            for ch in range(NCHUNK):
                xt = xpool.tile([8, BLK * 4 * BC], FP16, tag=f"xq{ch}",
                                name=f"xq{ch}_{blk}")
                nc.sync.dma_start(out=xt, in_=d["xq"][ch, blk])
                xtiles[ch][blk] = xt

        fetch(0)
        fetch(1)

        s4s = [None, None]
        sos = [None, None]
        pps = [None, None]

        def front(s, ch):
            """input/bias injection + 4 gate MMs + merged sigmoids.

            Phase P: bias+input via one K=8 block-diagonal MM (start=True
            clears the bank) then 4 recurrent MMs.
            Phase H: telescoping accumulation -- the PSUM bank is never
            cleared after its init step; each step adds W.h(s-1) and
            subtracts W.h(s-3) (the identical product from two steps ago on
            this parity bank, cancelling exactly up to f32 rounding), so the
            constant bias persists with no 512-column injection stream.
            """
            phase_p = s < NP
            gates = gpool.tile([H, 4 * BC], F32, tag=f"g{ch}",
                               name=f"g{ch}_{s}")
            if phase_p:
                blk, sl = divmod(s, BLK)
                rhs = xtiles[ch][blk][:, sl * 4 * BC:(sl + 1) * 4 * BC]
                nc.tensor.matmul(gates, bp8, rhs, start=True, stop=False,
                                 skip_group_check=True)
            elif s < NP + 2:
                nc.tensor.matmul(gates, bh4, ones4, start=True, stop=False,
                                 skip_group_check=True)
            else:
                hm3 = hist[ch][:, ((s - 3) % 8) * BC: ((s - 3) % 8 + 1) * BC]
                for j in range(4):
                    nc.tensor.matmul(gates[:, j * H:(j + 1) * H],
                                     whhT_hn[:, j * H:(j + 1) * H], hm3,
                                     start=False, stop=False,
                                     skip_group_check=True)
            whh = whhT_p if phase_p else whhT_h
            hprev = hist[ch][:, ((s - 1) % 8) * BC: ((s - 1) % 8 + 1) * BC]
            for j in range(4):
                nc.tensor.matmul(gates[:, j * H:(j + 1) * H],
                                 whh[:, j * H:(j + 1) * H], hprev,
                                 start=False, stop=(j == 3),
                                 skip_group_check=True)
            s4 = spool.tile([H, 3 * BC], FP16, tag=f"s4{ch}",
                            name=f"s4{ch}_{s}")
            nc.scalar.activation(s4, gates[:, 0:3 * H], AF.Sigmoid)
            so = spool.tile([H, BC], FP16, tag=f"so{ch}", name=f"so{ch}_{s}")
            nc.scalar.activation(so, gates[:, 3 * H:4 * H], AF.Sigmoid)
            s4s[ch] = s4
            sos[ch] = so

        def back(s, ch):
            """cell update on DVE + sigma(cT) + hT + batched prediction MM."""
            s4 = s4s[ch]
            t2 = wpool.tile([H, BC], FP16, tag=f"t2{ch}", name=f"t2{ch}_{s}")
            nc.vector.tensor_mul(t2, s4[:, H:2 * H], cT[ch])
            u = wpool.tile([H, BC], FP16, tag=f"u{ch}", name=f"u{ch}_{s}")
            nc.vector.scalar_tensor_tensor(u, s4[:, 2 * H:3 * H], 0.5,
                                           s4[:, 0:H], OP.subtract, OP.mult)
            nc.vector.scalar_tensor_tensor(cT[ch], u, 4.0, t2,
                                           OP.mult, OP.add)
            tc_ = wpool.tile([H, BC], FP16, tag=f"sc{ch}", name=f"sc{ch}_{s}")
            nc.scalar.activation(tc_, cT[ch], AF.Tanh, scale=0.5)
            hslot = hist[ch][:, (s % 8) * BC: (s % 8 + 1) * BC]
            nc.vector.tensor_mul(hslot, tc_, sos[ch])

            # Predictions for group G (steps 4G..4G+3) are emitted one step
            # AFTER the group completes (s = 4G+4): by then every hist slot
            # the matmul reads is already written, so the in-order PE queue
            # never stalls on it (the 8-slot ring gives a 4-step reuse gap).
            if s % 4 == 0 and s >= 4:
                emit_pred(s // 4 - 1, ch)

        def emit_pred(G, ch):
            NG = (NT + 3) // 4
            r = G % 32
            n = min(NT - 4 * G, 4) * BC
            base = (G % 2) * 4 * BC
            if r == 0:
                pps[ch] = ppool.tile([H, 4 * BC], F32, tag=f"pps{ch}",
                                     name=f"pps{ch}_{G}")
            nc.tensor.matmul(pps[ch][:, 0:n],
                             woutZ[:, H - r: 2 * H - r],
                             hist[ch][:, base: base + n],
                             start=(r == 0), stop=(r == 31 or G == NG - 1),
                             skip_group_check=True)
            if r == 31 or G == NG - 1:
                e = G // 32
                pc = wpool.tile([32, 4 * BC], F32, tag=f"pc{ch}",
                                name=f"pc{ch}_{G}")
                nc.vector.tensor_copy(pc, pps[ch][0:32, :])
                nc.sync.dma_start(out=d["preds"][e, ch], in_=pc)

        # Software pipeline: full A-step then full B-step per iteration.
        # Each engine's FIFO then alternates A-stage / B-stage, which locks
        # the two chunks half a step out of phase (emitting both fronts
        # together lets the chunks drift in-phase and exposes the full
        # serial chain latency).
        for s in range(NT):
            if s % BLK == BLK // 2:
                nb = s // BLK + 2
                if nb < NBLK:
                    fetch(nb)
            front(s, 0)
            back(s, 0)
            front(s, 1)
            back(s, 1)

        # final prediction group(s) not yet emitted by the loop
        NG = (NT + 3) // 4
        emit_pred(NG - 1, 0)
        emit_pred(NG - 1, 1)


@functools.lru_cache(maxsize=2)
def _program(NP, NH):
    nc = bacc.Bacc("TRN2", target_bir_lowering=False, debug=False,
                   num_devices=NCORES)
    NT = NP + NH
    NEP = (NT + 127) // 128
    NBLK = (NP + BLK - 1) // BLK
    d = {
        "whhT_p": nc.dram_tensor("whhT_p", [H, 4 * H], FP16,
                                 kind="ExternalInput").ap(),
        "whhT_h": nc.dram_tensor("whhT_h", [H, 4 * H], FP16,
                                 kind="ExternalInput").ap(),
        "whhT_hn": nc.dram_tensor("whhT_hn", [H, 4 * H], FP16,
                                  kind="ExternalInput").ap(),
        "bp8": nc.dram_tensor("bp8", [8, H], FP16, kind="ExternalInput").ap(),
        "bh4": nc.dram_tensor("bh4", [4, H], FP16, kind="ExternalInput").ap(),
        "ones4": nc.dram_tensor("ones4", [4, 4 * BC], FP16,
                                kind="ExternalInput").ap(),
        "woutZ": nc.dram_tensor("woutZ", [H, 2 * H], FP16,
                                kind="ExternalInput").ap(),
        "xq": nc.dram_tensor("xq", [NCHUNK, NBLK, 8, BLK * 4 * BC], FP16,
                             kind="ExternalInput").ap(),
        "preds": nc.dram_tensor("preds", [NEP, NCHUNK, 32, 4 * BC], F32,
                                kind="ExternalOutput").ap(),
    }
    with tile.TileContext(nc) as tc:
        _build_body(tc, d, NP, NH)
    nc.compile()
    return nc


def _host_prep(y_flow, W_ih, W_hh, b_ih, b_hh, W_out, b_out, NP):
    """Build per-core input maps. y_flow: (B, T, 1) f32."""
    f16 = np.float16
    W_ih = np.asarray(W_ih, np.float32)
    W_hh = np.asarray(W_hh, np.float32)
    W_out = np.asarray(W_out, np.float32)
    bias = np.asarray(b_ih, np.float32) + np.asarray(b_hh, np.float32)
    b_out = np.asarray(b_out, np.float32)

    W_eff = W_hh + W_ih @ W_out           # [4H, H] (phase-H feedback fold)
    b_eff = bias + W_ih[:, 0] * b_out[0]

    sc = _SCALE[:, None]
    whhT_p = np.ascontiguousarray((W_hh[_PERM] * sc).T).astype(f16)
    whhT_h = np.ascontiguousarray((W_eff[_PERM] * sc).T).astype(f16)
    whhT_hn = (-whhT_h).astype(f16)

    wih_s = (W_ih[_PERM, 0] * _SCALE_B).astype(np.float32)
    b_s = (bias[_PERM] * _SCALE_B).astype(np.float32)
    beff_s = (b_eff[_PERM] * _SCALE_B).astype(np.float32)

    bp8 = np.zeros((8, H), np.float32)
    bh4 = np.zeros((4, H), np.float32)
    ones4 = np.zeros((4, 4 * BC), np.float32)
    for j in range(4):
        bp8[2 * j] = wih_s[j * H:(j + 1) * H]
        bp8[2 * j + 1] = b_s[j * H:(j + 1) * H]
        bh4[j] = beff_s[j * H:(j + 1) * H]
        ones4[j, j * BC:(j + 1) * BC] = 1.0

    woutZ = np.zeros((H, 2 * H), np.float32)
    woutZ[:, H] = W_out[0]

    NBLK = (NP + BLK - 1) // BLK
    NPAD = NBLK * BLK
    y = np.asarray(y_flow, np.float32)[:, :, 0]                   # [B, T]
    in_maps = []
    for core in range(NCORES):
        yc = y[core * BS:(core + 1) * BS]                         # [BS, T]
        xq = np.zeros((NCHUNK, NPAD, 8, 4 * BC), np.float32)
        for ch in range(NCHUNK):
            ystep = yc[ch * BC:(ch + 1) * BC, :NP].T              # [NP, BC]
            for j in range(4):
                xq[ch, :NP, 2 * j, j * BC:(j + 1) * BC] = ystep
                xq[ch, :, 2 * j + 1, j * BC:(j + 1) * BC] = 1.0
        # [ch, NBLK, BLK, 8, 512] -> [ch, NBLK, 8, BLK*512]
        xq = xq.reshape(NCHUNK, NBLK, BLK, 8, 4 * BC)
        xq = np.ascontiguousarray(xq.transpose(0, 1, 3, 2, 4))
        xq = xq.reshape(NCHUNK, NBLK, 8, BLK * 4 * BC)
        in_maps.append({
            "whhT_p": whhT_p, "whhT_h": whhT_h, "whhT_hn": whhT_hn,
            "bp8": bp8.astype(f16), "bh4": bh4.astype(f16),
            "ones4": ones4.astype(f16), "woutZ": woutZ.astype(f16),
            "xq": xq.astype(f16),
        })
    return in_maps


def kernel(y_flow, x_dyn, W_ih, W_hh, b_ih, b_hh, W_out, b_out, twin_idx,
           _trace=False):
    twin = int(twin_idx)
    assert twin == 256, f"kernel hardcodes twin_idx=256, got {twin}"
    B, T, _ = y_flow.shape
    assert (B, T) == (2048, 512)
    NP, NH = twin - 1, T - twin
    NT = NP + NH

    nc = _program(NP, NH)
    in_maps = _host_prep(y_flow, W_ih, W_hh, b_ih, b_hh, W_out, b_out, NP)
    res = run_bass_kernel_spmd(nc, in_maps, core_ids=list(range(NCORES)),
                               trace=_trace)

    b_out = np.asarray(b_out, np.float32)
    out = np.empty((B, NT, 1), np.float32)
    for core in range(NCORES):
        p = np.asarray(res.results[core]["preds"], np.float32)
        nep = p.shape[0]
        a = p.reshape(nep, NCHUNK, 32, 4, BC)      # [e, ch, r, j, b]
        for ch in range(NCHUNK):
            blk = a[:, ch].transpose(3, 0, 1, 2).reshape(BC, -1)[:, :NT]
            out[core * BS + ch * BC: core * BS + (ch + 1) * BC, :, 0] = \
                blk + b_out[0]
    if _trace:
        kernel._last_results = res
    return out


# revision 34
# speedup vs baseline: 1.1626x; 1.1626x over previous
"""Bass/Trainium2 kernel for nn_BaselineLSTM (B=2048, T=512, H=128, twin=256).

Strategy (v2):
  - Data-parallel: batch 2048 -> 8 cores x 256; each core runs 2 interleaved
    chunks of 128 batch (pipelining hides per-step cross-engine latency).
  - State kept transposed: h/c = [H=128 partitions, batch free]; state
    variables are scaled: hT = h/2, cT = 2c, so that every tanh can be
    computed as a sigmoid and all fix-up constants fold into weights:
      tanh(x) = 2*sigmoid(2x) - 1.
  - ONE sigmoid ACT per chunk-step covers all four gates [i|f|o|g]: the
    g-block rows of the stationary weights are pre-scaled so the matmul
    emits 2*pre_g there; a second small sigmoid covers sigma(cT)=sigma(2c).
  - Input + bias enter via ONE K=8 (phase P) / K=4 (phase H) matmul with a
    block-diagonal rhs (phase P rhs streamed from DRAM, phase H rhs static),
    accumulated into the gates PSUM bank before the 4 recurrent matmuls.
  - Cell update on DVE only (gpsimd is pathologically slow for elementwise):
      t2 = sf*cT;  u = (s2g-0.5)*si;  cT = 4u + t2       (scalar_tensor_tensor)
      hT = (sigma(cT)-0.5)*so                             (scalar_tensor_tensor)
  - fp16 everywhere on-chip (not bf16): the 2*sigmoid(2x)-1 rewrite loses
    absolute precision near 0.5 in bf16; fp16's 10 mantissa bits restore it,
    and fp16 keeps the DVE 2x/4x packed modes.
  - Predictions p_t = (2*W_out) hT_t (+ b_out on host): hT kept in a 4-slot
    ring; one shifted-stationary matmul per 4 steps accumulates 128 steps
    into one PSUM bank, flushed to DRAM per 128-step epoch.
"""

import functools

import numpy as np

import concourse.bacc as bacc
import concourse.tile as tile
from concourse import mybir
from concourse.bass_utils import run_bass_kernel_spmd

F32 = mybir.dt.float32
FP16 = mybir.dt.float16
AF = mybir.ActivationFunctionType
OP = mybir.AluOpType

H = 128          # hidden
NCORES = 8
BS = 256         # batch per core
BC = 128         # batch per chunk
NCHUNK = 2
BLK = 32         # xq steps per DMA block

# kernel gate order == pytorch order (i, f, g, o): sigma(i,f,g) is one
# contiguous on-chain activation; sigma(o) is separate and off-chain (o is
# first needed only after tanh(c)).
_PERM = np.arange(512)
# g-gate rows doubled so the matmul emits 2*pre_g for the
# tanh(x) = 2*sigmoid(2x)-1 rewrite; cT state = 2c.
_SCALE = np.repeat([1.0, 1.0, 2.0, 1.0], 128)
_SCALE_B = _SCALE


def _build_body(tc, d, NP, NH):
    nc = tc.nc
    NT = NP + NH
    NBLK = (NP + BLK - 1) // BLK

    import contextlib
    with contextlib.ExitStack() as ctx:
        consts = ctx.enter_context(tc.tile_pool(name="consts", bufs=1))
        state = ctx.enter_context(tc.tile_pool(name="state", bufs=1))
        spool = ctx.enter_context(tc.tile_pool(name="sig", bufs=3))
        wpool = ctx.enter_context(tc.tile_pool(name="work", bufs=3))
        xpool = ctx.enter_context(tc.tile_pool(name="xq", bufs=2))
        gpool = ctx.enter_context(tc.tile_pool(name="gates", bufs=3, space="PSUM"))
        ppool = ctx.enter_context(tc.tile_pool(name="ppsum", bufs=1, space="PSUM"))

        # ---- constants to SBUF
        whhT_p = consts.tile([H, 4 * H], FP16, tag="whhT_p")
        whhT_h = consts.tile([H, 4 * H], FP16, tag="whhT_h")
        bp8 = consts.tile([8, H], FP16, tag="bp8")
        bh4 = consts.tile([4, H], FP16, tag="bh4")
        ones4 = consts.tile([4, 4 * BC], FP16, tag="ones4")
        woutZ = consts.tile([H, 2 * H], FP16, tag="woutZ")
        nc.sync.dma_start(out=whhT_p, in_=d["whhT_p"])
        nc.sync.dma_start(out=whhT_h, in_=d["whhT_h"])
        nc.sync.dma_start(out=bp8, in_=d["bp8"])
        nc.sync.dma_start(out=bh4, in_=d["bh4"])
        nc.sync.dma_start(out=ones4, in_=d["ones4"])
        nc.sync.dma_start(out=woutZ, in_=d["woutZ"])

        # ---- state
        hist = []
        cT = []
        for ch in range(NCHUNK):
            hh = state.tile([H, 8 * BC], FP16, tag=f"hist{ch}")
            c = state.tile([H, BC], FP16, tag=f"cT{ch}")
            nc.vector.memset(hh, 0.0)
            nc.vector.memset(c, 0.0)
            hist.append(hh)
            cT.append(c)

        # ---- xq stream (phase P block-diag rhs), double buffered
        xtiles = [[None] * NBLK for _ in range(NCHUNK)]

        def fetch(blk):
            for ch in range(NCHUNK):
                xt = xpool.tile([8, BLK * 4 * BC], FP16, tag=f"xq{ch}",
                                name=f"xq{ch}_{blk}")
                nc.sync.dma_start(out=xt, in_=d["xq"][ch, blk])
                xtiles[ch][blk] = xt

        fetch(0)
        fetch(1)

        s4s = [None, None]
        sos = [None, None]
        pps = [None, None]

        def front(s, ch):
            """input/bias MM + 4 gate MMs + merged sigmoids."""
            phase_p = s < NP
            gates = gpool.tile([H, 4 * BC], F32, tag=f"g{ch}",
                               name=f"g{ch}_{s}")
            if phase_p:
                blk, sl = divmod(s, BLK)
                rhs = xtiles[ch][blk][:, sl * 4 * BC:(sl + 1) * 4 * BC]
                nc.tensor.matmul(gates, bp8, rhs, start=True, stop=False,
                                 skip_group_check=True)
            else:
                nc.tensor.matmul(gates, bh4, ones4, start=True, stop=False,
                                 skip_group_check=True)
            whh = whhT_p if phase_p else whhT_h
            hprev = hist[ch][:, ((s - 1) % 8) * BC: ((s - 1) % 8 + 1) * BC]
            for j in range(4):
                nc.tensor.matmul(gates[:, j * H:(j + 1) * H],
                                 whh[:, j * H:(j + 1) * H], hprev,
                                 start=False, stop=(j == 3),
                                 skip_group_check=True)
            s4 = spool.tile([H, 3 * BC], FP16, tag=f"s4{ch}",
                            name=f"s4{ch}_{s}")
            nc.scalar.activation(s4, gates[:, 0:3 * H], AF.Sigmoid)
            so = spool.tile([H, BC], FP16, tag=f"so{ch}", name=f"so{ch}_{s}")
            nc.scalar.activation(so, gates[:, 3 * H:4 * H], AF.Sigmoid)
            s4s[ch] = s4
            sos[ch] = so

        def back(s, ch):
            """cell update on DVE + sigma(cT) + hT + batched prediction MM."""
            s4 = s4s[ch]
            t2 = wpool.tile([H, BC], FP16, tag=f"t2{ch}", name=f"t2{ch}_{s}")
            nc.vector.tensor_mul(t2, s4[:, H:2 * H], cT[ch])
            u = wpool.tile([H, BC], FP16, tag=f"u{ch}", name=f"u{ch}_{s}")
            nc.vector.scalar_tensor_tensor(u, s4[:, 2 * H:3 * H], 0.5,
                                           s4[:, 0:H], OP.subtract, OP.mult)
            nc.vector.scalar_tensor_tensor(cT[ch], u, 4.0, t2,
                                           OP.mult, OP.add)
            tc_ = wpool.tile([H, BC], FP16, tag=f"sc{ch}", name=f"sc{ch}_{s}")
            nc.scalar.activation(tc_, cT[ch], AF.Tanh, scale=0.5)
            hslot = hist[ch][:, (s % 8) * BC: (s % 8 + 1) * BC]
            nc.vector.tensor_mul(hslot, tc_, sos[ch])

            # Predictions for group G (steps 4G..4G+3) are emitted one step
            # AFTER the group completes (s = 4G+4): by then every hist slot
            # the matmul reads is already written, so the in-order PE queue
            # never stalls on it (the 8-slot ring gives a 4-step reuse gap).
            if ch == 0 and s % 4 == 0 and s >= 4:
                emit_pred(s // 4 - 1, 0)
            if ch == 1 and s % 4 == 2 and s >= 6:
                emit_pred((s - 2) // 4 - 1, 1)

        def emit_pred(G, ch):
            NG = (NT + 3) // 4
            r = G % 32
            n = min(NT - 4 * G, 4) * BC
            base = (G % 2) * 4 * BC
            if r == 0:
                pps[ch] = ppool.tile([H, 4 * BC], F32, tag=f"pps{ch}",
                                     name=f"pps{ch}_{G}")
            nc.tensor.matmul(pps[ch][:, 0:n],
                             woutZ[:, H - r: 2 * H - r],
                             hist[ch][:, base: base + n],
                             start=(r == 0), stop=(r == 31 or G == NG - 1),
                             skip_group_check=True)
            if r == 31 or G == NG - 1:
                e = G // 32
                pc = wpool.tile([32, 4 * BC], F32, tag=f"pc{ch}",
                                name=f"pc{ch}_{G}")
                nc.vector.tensor_copy(pc, pps[ch][0:32, :])
                nc.sync.dma_start(out=d["preds"][e, ch], in_=pc)

        # Software pipeline: full A-step then full B-step per iteration.
        # Each engine's FIFO then alternates A-stage / B-stage, which locks
        # the two chunks half a step out of phase (emitting both fronts
        # together lets the chunks drift in-phase and exposes the full
        # serial chain latency).
        for s in range(NT):
            if s % BLK == BLK // 2:
                nb = s // BLK + 2
                if nb < NBLK:
                    fetch(nb)
            front(s, 0)
            back(s, 0)
            front(s, 1)
            back(s, 1)

        # final prediction group(s) not yet emitted by the loop
        NG = (NT + 3) // 4
        emit_pred(NG - 1, 0)
        emit_pred(NG - 1, 1)


@functools.lru_cache(maxsize=2)
def _program(NP, NH):
    nc = bacc.Bacc("TRN2", target_bir_lowering=False, debug=False,
                   num_devices=NCORES)
    NT = NP + NH
    NEP = (NT + 127) // 128
    NBLK = (NP + BLK - 1) // BLK
    d = {
        "whhT_p": nc.dram_tensor("whhT_p", [H, 4 * H], FP16,
                                 kind="ExternalInput").ap(),
        "whhT_h": nc.dram_tensor("whhT_h", [H, 4 * H], FP16,
                                 kind="ExternalInput").ap(),
        "bp8": nc.dram_tensor("bp8", [8, H], FP16, kind="ExternalInput").ap(),
        "bh4": nc.dram_tensor("bh4", [4, H], FP16, kind="ExternalInput").ap(),
        "ones4": nc.dram_tensor("ones4", [4, 4 * BC], FP16,
                                kind="ExternalInput").ap(),
        "woutZ": nc.dram_tensor("woutZ", [H, 2 * H], FP16,
                                kind="ExternalInput").ap(),
        "xq": nc.dram_tensor("xq", [NCHUNK, NBLK, 8, BLK * 4 * BC], FP16,
                             kind="ExternalInput").ap(),
        "preds": nc.dram_tensor("preds", [NEP, NCHUNK, 32, 4 * BC], F32,
                                kind="ExternalOutput").ap(),
    }
    with tile.TileContext(nc) as tc:
        _build_body(tc, d, NP, NH)
    nc.compile()
    return nc


def _host_prep(y_flow, W_ih, W_hh, b_ih, b_hh, W_out, b_out, NP):
    """Build per-core input maps. y_flow: (B, T, 1) f32."""
    f16 = np.float16
    W_ih = np.asarray(W_ih, np.float32)
    W_hh = np.asarray(W_hh, np.float32)
    W_out = np.asarray(W_out, np.float32)
    bias = np.asarray(b_ih, np.float32) + np.asarray(b_hh, np.float32)
    b_out = np.asarray(b_out, np.float32)

    W_eff = W_hh + W_ih @ W_out           # [4H, H] (phase-H feedback fold)
    b_eff = bias + W_ih[:, 0] * b_out[0]

    sc = _SCALE[:, None]
    whhT_p = np.ascontiguousarray((W_hh[_PERM] * sc).T).astype(f16)
    whhT_h = np.ascontiguousarray((W_eff[_PERM] * sc).T).astype(f16)

    wih_s = (W_ih[_PERM, 0] * _SCALE_B).astype(np.float32)
    b_s = (bias[_PERM] * _SCALE_B).astype(np.float32)
    beff_s = (b_eff[_PERM] * _SCALE_B).astype(np.float32)

    bp8 = np.zeros((8, H), np.float32)
    bh4 = np.zeros((4, H), np.float32)
    ones4 = np.zeros((4, 4 * BC), np.float32)
    for j in range(4):
        bp8[2 * j] = wih_s[j * H:(j + 1) * H]
        bp8[2 * j + 1] = b_s[j * H:(j + 1) * H]
        bh4[j] = beff_s[j * H:(j + 1) * H]
        ones4[j, j * BC:(j + 1) * BC] = 1.0

    woutZ = np.zeros((H, 2 * H), np.float32)
    woutZ[:, H] = W_out[0]

    NBLK = (NP + BLK - 1) // BLK
    NPAD = NBLK * BLK
    y = np.asarray(y_flow, np.float32)[:, :, 0]                   # [B, T]
    in_maps = []
    for core in range(NCORES):
        yc = y[core * BS:(core + 1) * BS]                         # [BS, T]
        xq = np.zeros((NCHUNK, NPAD, 8, 4 * BC), np.float32)
        for ch in range(NCHUNK):
            ystep = yc[ch * BC:(ch + 1) * BC, :NP].T              # [NP, BC]
            for j in range(4):
                xq[ch, :NP, 2 * j, j * BC:(j + 1) * BC] = ystep
                xq[ch, :, 2 * j + 1, j * BC:(j + 1) * BC] = 1.0
        # [ch, NBLK, BLK, 8, 512] -> [ch, NBLK, 8, BLK*512]
        xq = xq.reshape(NCHUNK, NBLK, BLK, 8, 4 * BC)
        xq = np.ascontiguousarray(xq.transpose(0, 1, 3, 2, 4))
        xq = xq.reshape(NCHUNK, NBLK, 8, BLK * 4 * BC)
        in_maps.append({
            "whhT_p": whhT_p, "whhT_h": whhT_h,
            "bp8": bp8.astype(f16), "bh4": bh4.astype(f16),
            "ones4": ones4.astype(f16), "woutZ": woutZ.astype(f16),
            "xq": xq.astype(f16),
        })
    return in_maps


def kernel(y_flow, x_dyn, W_ih, W_hh, b_ih, b_hh, W_out, b_out, twin_idx,
           _trace=False):
    twin = int(twin_idx)
    assert twin == 256, f"kernel hardcodes twin_idx=256, got {twin}"
    B, T, _ = y_flow.shape
    assert (B, T) == (2048, 512)
    NP, NH = twin - 1, T - twin
    NT = NP + NH

    nc = _program(NP, NH)
    in_maps = _host_prep(y_flow, W_ih, W_hh, b_ih, b_hh, W_out, b_out, NP)
    res = run_bass_kernel_spmd(nc, in_maps, core_ids=list(range(NCORES)),
                               trace=_trace)

    b_out = np.asarray(b_out, np.float32)
    out = np.empty((B, NT, 1), np.float32)
    for core in range(NCORES):
        p = np.asarray(res.results[core]["preds"], np.float32)
        nep = p.shape[0]
        a = p.reshape(nep, NCHUNK, 32, 4, BC)      # [e, ch, r, j, b]
        for ch in range(NCHUNK):
            blk = a[:, ch].transpose(3, 0, 1, 2).reshape(BC, -1)[:, :NT]
            out[core * BS + ch * BC: core * BS + (ch + 1) * BC, :, 0] = \
                blk + b_out[0]
    if _trace:
        kernel._last_results = res
    return out
